# revision 8
# baseline (speedup 1.0000x reference)
"""Trainium2 Bass kernel: paged int8-KV-cache GQA decode attention, 8-core SPMD.

Contract: kernel(**inputs) takes the FULL unsharded numpy inputs (as produced by
the reference setup_inputs) and returns the FULL [32, 4096] float32 output.

Strategy (data parallel + split-K over token windows):
  - Work units are (sequence, token-window) RANGES, flash-decoding style:
    every core runs an identical program over R ranges of compile-time tile
    counts rs[i]; a host-side solver cuts the 32 sequences' token streams
    into 8 windows per range size so that padding is ~the global remainder
    (<1%), vs ~12% for whole-sequence slotting.  Per-range partial (PV, Z)
    pairs are summed per sequence on the host (exact: no max-subtraction is
    used, so partial softmax sums add linearly).
  - K/V int8 cache values are gathered per block_tables into per-core packed
    int8 buffers (1 byte per element in HBM), laid out block-major so every
    HBM->SBUF DMA is one contiguous run per partition.  SWDGE DMAs cast
    int8 -> bf16 inline (exact: values are +-127 integers).
  - Work is chopped into <=BT-token-tile blocks per (range, kvh-group); a
    tiny lead block primes the pipeline so the PE starts ~2us in.
  - Per block: QK matmuls (K^T tile as stationary operand) -> one DVE mul by
    k_scale*softmax_scale -> one ACT exp -> one DVE mul by v_scale; then the
    PV+Z fused matmul lhsT=[e|ev], rhs=[V|mask] accumulated into a per-
    (range,group) PSUM bank, kvh j2 on PE column-group j2 (tile_position).
    Rows 32*j2+0..3 hold Z (col 128); rows 32*j2+4..7 hold PV (cols 0..127).
  Softmax skips max-subtraction (scores are O(20) at most; fp32 exp is safe).
"""

import math
import random
import sys
from contextlib import ExitStack

import numpy as np

sys.path.insert(0, "/opt/trn_rl_repo")

import ml_dtypes  # noqa: E402

import concourse.bass as bass  # noqa: E402
import concourse.mybir as mybir  # noqa: E402
import concourse.tile as tile  # noqa: E402
from concourse import bacc  # noqa: E402
from concourse.bass_utils import run_bass_kernel_spmd  # noqa: E402

BF16 = ml_dtypes.bfloat16

B = 32
NUM_HEADS = 32
KVH = 8
D = 128
REP = NUM_HEADS // KVH  # 4
BLOCK_SIZE = 256
T = 4096
P = 128
DV = D + 1  # V columns + mask column
SCALE = 1.0 / float(np.sqrt(D))
NCORES = 8
BT = 12     # token tiles per pipeline block


# ---------------------------------------------------------------------------
# host-side planning
# ---------------------------------------------------------------------------

def _greedy_assign(sizes, tiles):
    """Cut sequences' tile tails into 8 windows per range size (desc order).

    Returns (pad, plan) where plan[range_index] is a list of up to 8
    (seq, w0_tile, take) triples (core order), or None entries.
    """
    rem = [(int(t), b) for b, t in enumerate(tiles)]
    pad = 0
    order = np.argsort([-s for s in sizes], kind="stable")
    plan = [[None] * NCORES for _ in sizes]
    for ri in order:
        r = sizes[ri]
        for c in range(NCORES):
            rem.sort(reverse=True)
            t0, b = rem[0]
            if t0 == 0:
                pad += r
                continue
            take = min(r, t0)
            pad += r - take
            rem[0] = (t0 - take, b)
            plan[ri][c] = (b, t0 - take, take)
    left = sum(t for t, b in rem)
    return (pad if left == 0 else None), plan


def _plan(context_lens):
    """Choose shared range sizes + (core, range) -> (seq, window) assignment.

    Padding is exactly NCORES*sum(rs) - total for any feasible plan, so
    search ascending per-core totals and take the first feasible config.
    """
    tiles = [int(math.ceil(int(c) / P)) for c in context_lens]
    total = sum(tiles)
    lo = (total + NCORES - 1) // NCORES
    rng = random.Random(0)
    for tot in range(lo, lo + 13):
        for R in range(4, 11):
            if R > tot:
                break
            for _ in range(3000):
                cuts = sorted(rng.sample(range(1, tot), R - 1)) if R > 1 else []
                s = [b - a for a, b in zip([0] + cuts, cuts + [tot])]
                if max(s) > 32:
                    continue
                s.sort(reverse=True)
                pad, plan = _greedy_assign(s, tiles)
                if pad is not None:
                    # ascending size order: tiny ranges prime the pipeline,
                    # big trailing blocks keep the DMA queue fed at the end
                    pairs = sorted(zip(s, plan), key=lambda x: x[0])
                    return [p[0] for p in pairs], [p[1] for p in pairs]
    raise AssertionError("range planner failed")


def _blocks(rs):
    """[(ri, g, tile_off_in_range, bt)] pipeline blocks."""
    out = []
    for ri, n in enumerate(rs):
        for g in range(2):
            bo = 0
            while bo < n:
                bt = min(BT, n - bo)
                out.append((ri, g, bo, bt))
                bo += bt
    return out


def _quantize(x):
    absmax = np.abs(x).max(axis=-1)
    scale = np.where(absmax > 0.0, absmax / 127.0, 1.0).astype(np.float32)
    xq = np.clip(np.round(x / scale[..., None]), -127.0, 127.0).astype(np.int8)
    return xq, scale


def _pack_inputs(inputs, rs, plan, blocks):
    q = inputs["q"].reshape(B, NUM_HEADS, D).astype(np.float32)
    k = inputs["k"].reshape(B, KVH, D).astype(np.float32)
    v = inputs["v"].reshape(B, KVH, D).astype(np.float32)
    kc = np.ascontiguousarray(
        inputs["k_cache_q"].reshape(-1, KVH, D).astype(np.int8))
    vc = np.ascontiguousarray(
        inputs["v_cache_q"].reshape(-1, KVH, D).astype(np.int8))
    ks = np.ascontiguousarray(inputs["k_scale"].reshape(-1, KVH)).astype(np.float32)
    vs = np.ascontiguousarray(inputs["v_scale"].reshape(-1, KVH)).astype(np.float32)
    bt_tab = inputs["block_tables"]
    ctx = inputs["context_lens"]
    sm = inputs["slot_mapping"]

    # store_kvcache_int8: quantize the new token and scatter into the cache
    kq, ksn = _quantize(k)
    vq, vsn = _quantize(v)
    kc = kc.copy(); vc = vc.copy(); ks = ks.copy(); vs = vs.copy()
    kc[sm] = kq; vc[sm] = vq; ks[sm] = ksn; vs[sm] = vsn

    R = len(rs)
    RT = sum(rs)
    offs = np.concatenate([[0], np.cumsum(rs)])
    KSZ = sum(bt * P * 4 * D for (_, _, _, bt) in blocks)     # int8 elems
    VSZ = sum(bt * 4 * P * DV for (_, _, _, bt) in blocks)

    # gather + zero-pad each sequence once, globally
    kg_all = {}; vg_all = {}; ksg_all = {}; vsg_all = {}
    for b in range(B):
        nt = int(math.ceil(int(ctx[b]) / P)) * P
        cl = int(ctx[b])
        flat = (bt_tab[b][:, None] * BLOCK_SIZE
                + np.arange(BLOCK_SIZE, dtype=np.int64)[None, :]).reshape(-1)[:nt]
        valid = (np.arange(nt) < cl)
        kg_all[b] = kc[flat] * valid[:, None, None]          # [nt, KVH, D]
        vg = vc[flat] * valid[:, None, None]
        n = nt // P
        vgm = np.zeros((n, P, KVH, DV), dtype=np.int8)
        vgm[:, :, :, :D] = vg.reshape(n, P, KVH, D)
        vgm[:, :, :, D] = valid.reshape(n, P)[:, :, None]
        vg_all[b] = vgm
        ksg_all[b] = (ks[flat] * SCALE) * valid[:, None]     # [nt, KVH]
        vsg_all[b] = vs[flat] * valid[:, None]

    in_maps = []
    for c in range(NCORES):
        kt_c = np.zeros((P, KSZ // P), dtype=np.int8)   # [d, flat]
        vp_c = np.zeros((P, VSZ // P), dtype=np.int8)   # [tok%128, flat]
        scb_c = np.zeros((P, 2, RT, 8), dtype=np.float32)
        qt_c = np.zeros((P, R * 32), dtype=BF16)
        # stage per-range gathered windows (padded to rs[ri] tiles)
        kw = {}; vw = {}
        for ri in range(R):
            n = rs[ri]
            o = int(offs[ri])
            w = plan[ri][c]
            kwin = np.zeros((n * P, KVH, D), dtype=np.int8)
            vwin = np.zeros((n, P, KVH, DV), dtype=np.int8)
            if w is not None:
                b, w0, take = w
                kwin[: take * P] = kg_all[b][w0 * P: (w0 + take) * P]
                vwin[: take] = vg_all[b][w0: w0 + take]
                ksgw = np.zeros((n * P, KVH), dtype=np.float32)
                vsgw = np.zeros((n * P, KVH), dtype=np.float32)
                ksgw[: take * P] = ksg_all[b][w0 * P: (w0 + take) * P]
                vsgw[: take * P] = vsg_all[b][w0 * P: (w0 + take) * P]
                scb_c[:, :, o: o + n, 0:4] = (
                    ksgw.reshape(n, P, 2, 4).transpose(1, 2, 0, 3))
                scb_c[:, :, o: o + n, 4:8] = (
                    vsgw.reshape(n, P, 2, 4).transpose(1, 2, 0, 3))
                qt_c[:, ri * 32: (ri + 1) * 32] = q[b].transpose(1, 0).astype(BF16)
            kw[ri] = kwin
            vw[ri] = vwin
        ko = vo = 0
        for (ri, g, bo, bt) in blocks:
            t0, t1 = bo * P, (bo + bt) * P
            kb = kw[ri][t0:t1, 4 * g: 4 * g + 4, :].transpose(2, 1, 0)  # [D,4,btP]
            ksz = 4 * bt * P
            kt_c[:, ko: ko + ksz] = kb.reshape(D, ksz)
            ko += ksz
            vb = vw[ri][bo: bo + bt, :, 4 * g: 4 * g + 4, :].transpose(1, 2, 0, 3)
            vsz = 4 * bt * DV
            vp_c[:, vo: vo + vsz] = vb.reshape(P, vsz)
            vo += vsz
        in_maps.append(dict(kt=kt_c, vp=vp_c, scb=scb_c, qt=qt_c))
    return in_maps


# ---------------------------------------------------------------------------
# device program
# ---------------------------------------------------------------------------

def _build_program(rs):
    blocks = _blocks(rs)
    R = len(rs)
    RT = sum(rs)
    offs = [0]
    for n in rs:
        offs.append(offs[-1] + n)
    KSZ = sum(bt * P * 4 * D for (_, _, _, bt) in blocks)
    VSZ = sum(bt * 4 * P * DV for (_, _, _, bt) in blocks)
    f32 = mybir.dt.float32
    bf16 = mybir.dt.bfloat16
    i8 = mybir.dt.int8
    EXP = mybir.ActivationFunctionType.Exp

    nc = bacc.Bacc("TRN2", target_bir_lowering=False, debug=False,
                   num_devices=NCORES)

    kt_d = nc.dram_tensor("kt", [P, KSZ // P], i8, kind="ExternalInput").ap()
    vp_d = nc.dram_tensor("vp", [P, VSZ // P], i8, kind="ExternalInput").ap()
    scb_d = nc.dram_tensor("scb", [P, 2, RT, 8], f32, kind="ExternalInput").ap()
    qt_d = nc.dram_tensor("qt", [P, R * 32], bf16, kind="ExternalInput").ap()
    out_d = nc.dram_tensor("out", [R, 2, P, DV], f32,
                           kind="ExternalOutput").ap()

    with tile.TileContext(nc) as tc, ExitStack() as ctx:
        const = ctx.enter_context(tc.tile_pool(name="const", bufs=1))
        kt_pool = ctx.enter_context(tc.tile_pool(name="ktp", bufs=6))
        v_pool = ctx.enter_context(tc.tile_pool(name="vpp", bufs=6))
        sc_pool = ctx.enter_context(tc.tile_pool(name="scp", bufs=2))
        work = ctx.enter_context(tc.tile_pool(name="wrk", bufs=3))
        o_pool = ctx.enter_context(tc.tile_pool(name="osb", bufs=2))
        ps_qk = ctx.enter_context(tc.tile_pool(name="psqk", bufs=3, space="PSUM"))
        ps_pv = ctx.enter_context(tc.tile_pool(name="pspv", bufs=2, space="PSUM"))

        qt = const.tile([P, R * 32], bf16)
        nc.sync.dma_start(qt, qt_d)

        ko = vo = 0
        cur = None
        pv = scb = None
        blk_i = 0
        nblk = {}
        for (s2, g2, _, _) in blocks:
            nblk[(s2, g2)] = nblk.get((s2, g2), 0) + 1
        for (ri, g, bo, bt) in blocks:
            n = rs[ri]
            o = offs[ri]
            if cur != (ri, g):
                cur = (ri, g)
                scb = sc_pool.tile([P, n, 8], f32, tag="scb")
                nc.sync.dma_start(scb, scb_d[:, g, o: o + n, :])
                pv = ps_pv.tile([P, DV], f32, tag="pv")
                nc.vector.memset(pv, 0.0)
                blk_i = 0
            ksz = 4 * bt * P
            kc = kt_pool.tile([P, 4, bt, P], bf16, tag="kt")
            nc.gpsimd.dma_start(
                kc, kt_d[:, ko: ko + ksz].rearrange(
                    "d (j i t) -> d j i t", j=4, i=bt))
            ko += ksz
            vsz = 4 * bt * DV
            vc = v_pool.tile([P, 4, bt, DV], bf16, tag="vt")
            nc.gpsimd.dma_start(
                vc, vp_d[:, vo: vo + vsz].rearrange(
                    "p (j i c) -> p j i c", j=4, i=bt))
            vo += vsz

            qk = ps_qk.tile([P, bt, 4, 4], f32, tag="qk")
            for i in range(bt):
                for j2 in range(4):
                    qcol = ri * 32 + (4 * g + j2) * 4
                    nc.tensor.matmul(
                        qk[:, i, j2, :],
                        lhsT=kc[:, j2, i, :],
                        rhs=qt[:, qcol: qcol + 4],
                        start=True, stop=True, skip_group_check=True)

            nc.vector.tensor_mul(
                qk, qk,
                scb[:, bo: bo + bt, 0:4].unsqueeze(3).to_broadcast(
                    [P, bt, 4, 4]))
            ew = work.tile([P, bt, 4, 8], bf16, tag="ew")
            nc.scalar.activation(ew[:, :, :, 0:4], qk, EXP)
            nc.vector.tensor_mul(
                ew[:, :, :, 4:8], ew[:, :, :, 0:4],
                scb[:, bo: bo + bt, 4:8].unsqueeze(3).to_broadcast(
                    [P, bt, 4, 4]))

            last = blk_i == nblk[cur] - 1
            for i in range(bt):
                for j2 in range(4):
                    nc.tensor.matmul(
                        pv[32 * j2: 32 * j2 + 8, :],
                        lhsT=ew[:, i, j2, :],
                        rhs=vc[:, j2, i, :],
                        start=(blk_i == 0 and i == 0),
                        stop=(last and i == bt - 1),
                        tile_position=(0, 32 * j2),
                        skip_group_check=True)
            blk_i += 1
            if last:
                osb = o_pool.tile([P, DV], f32, tag="osb")
                nc.vector.tensor_copy(osb, pv)
                nc.sync.dma_start(out_d[ri, g], osb)

    nc.compile()
    return nc


_PROGRAM_CACHE = {}
_PLAN_CACHE = {}


def _get_program(rs):
    key = tuple(rs)
    if key not in _PROGRAM_CACHE:
        _PROGRAM_CACHE[key] = _build_program(rs)
    return _PROGRAM_CACHE[key]


# ---------------------------------------------------------------------------
# entry point
# ---------------------------------------------------------------------------

def kernel(q, k, v, k_cache_q, v_cache_q, k_scale, v_scale,
           block_tables, context_lens, slot_mapping, _trace=False):
    inputs = dict(q=np.asarray(q), k=np.asarray(k), v=np.asarray(v),
                  k_cache_q=np.asarray(k_cache_q),
                  v_cache_q=np.asarray(v_cache_q),
                  k_scale=np.asarray(k_scale), v_scale=np.asarray(v_scale),
                  block_tables=np.asarray(block_tables),
                  context_lens=np.asarray(context_lens),
                  slot_mapping=np.asarray(slot_mapping))
    ctx_key = inputs["context_lens"].tobytes()
    if ctx_key not in _PLAN_CACHE:
        _PLAN_CACHE[ctx_key] = _plan(inputs["context_lens"])
    rs, plan = _PLAN_CACHE[ctx_key]
    blocks = _blocks(rs)
    in_maps = _pack_inputs(inputs, rs, plan, blocks)
    nc = _get_program(rs)
    res = run_bass_kernel_spmd(nc, in_maps, core_ids=list(range(NCORES)),
                               trace=_trace)

    num = np.zeros((B, NUM_HEADS, D), dtype=np.float64)
    den = np.zeros((B, NUM_HEADS), dtype=np.float64)
    R = len(rs)
    for c in range(NCORES):
        oc = res.results[c]["out"]  # [R, 2, P, DV] f32
        for ri in range(R):
            w = plan[ri][c]
            if w is None:
                continue
            b = w[0]
            for g in range(2):
                for j2 in range(4):
                    j = 4 * g + j2
                    den[b, 4 * j: 4 * j + 4] += oc[ri, g, 32 * j2: 32 * j2 + 4, D]
                    num[b, 4 * j: 4 * j + 4] += oc[ri, g,
                                                   32 * j2 + 4: 32 * j2 + 8, :D]
    out = (num / den[:, :, None]).astype(np.float32).reshape(B, NUM_HEADS * D)
    if _trace:
        return out, res
    return out


# revision 12
# speedup vs baseline: 1.0915x; 1.0915x over previous
"""Trainium2 Bass kernel: paged int8-KV-cache GQA decode attention, 8-core SPMD.

Contract: kernel(**inputs) takes the FULL unsharded numpy inputs (as produced by
the reference setup_inputs) and returns the FULL [32, 4096] float32 output.

Strategy (data parallel + split-K over token windows):
  - Work units are (sequence, token-window) RANGES, flash-decoding style:
    every core runs an identical program over R ranges of compile-time tile
    counts rs[i]; a host-side solver cuts the 32 sequences' token streams
    into 8 windows per range size so that padding is ~the global remainder
    (<1%), vs ~12% for whole-sequence slotting.  Per-range partial (PV, Z)
    pairs are summed per sequence on the host (exact: no max-subtraction is
    used, so partial softmax sums add linearly).
  - K/V int8 cache values are gathered per block_tables into per-core packed
    int8 buffers (1 byte per element in HBM), laid out block-major so every
    HBM->SBUF DMA is one contiguous run per partition.  SWDGE DMAs cast
    int8 -> bf16 inline (exact: values are +-127 integers).
  - Work is chopped into <=BT-token-tile blocks per (range, kvh-group); a
    tiny lead block primes the pipeline so the PE starts ~2us in.
  - Per block: QK matmuls (K^T tile as stationary operand) -> one DVE mul by
    k_scale*softmax_scale -> one ACT exp -> one DVE mul by v_scale; then the
    PV+Z fused matmul lhsT=[e|ev], rhs=[V|mask] accumulated into a per-
    (range,group) PSUM bank, kvh j2 on PE column-group j2 (tile_position).
    Rows 32*j2+0..3 hold Z (col 128); rows 32*j2+4..7 hold PV (cols 0..127).
  Softmax skips max-subtraction (scores are O(20) at most; fp32 exp is safe).
"""

import math
import random
import sys
from contextlib import ExitStack

import numpy as np

sys.path.insert(0, "/opt/trn_rl_repo")

import ml_dtypes  # noqa: E402

import concourse.bass as bass  # noqa: E402
import concourse.mybir as mybir  # noqa: E402
import concourse.tile as tile  # noqa: E402
from concourse import bacc  # noqa: E402
from concourse.bass_utils import run_bass_kernel_spmd  # noqa: E402

BF16 = ml_dtypes.bfloat16

B = 32
NUM_HEADS = 32
KVH = 8
D = 128
REP = NUM_HEADS // KVH  # 4
BLOCK_SIZE = 256
T = 4096
P = 128
DV = D + 1  # V columns + mask column
SCALE = 1.0 / float(np.sqrt(D))
NCORES = 8
BT = 12     # token tiles per pipeline block


# ---------------------------------------------------------------------------
# host-side planning
# ---------------------------------------------------------------------------

def _greedy_assign(sizes, tiles):
    """Cut sequences' tile tails into 8 windows per range size (desc order).

    Returns (pad, plan) where plan[range_index] is a list of up to 8
    (seq, w0_tile, take) triples (core order), or None entries.
    """
    rem = [(int(t), b) for b, t in enumerate(tiles)]
    pad = 0
    order = np.argsort([-s for s in sizes], kind="stable")
    plan = [[None] * NCORES for _ in sizes]
    for ri in order:
        r = sizes[ri]
        for c in range(NCORES):
            rem.sort(reverse=True)
            t0, b = rem[0]
            if t0 == 0:
                pad += r
                continue
            take = min(r, t0)
            pad += r - take
            rem[0] = (t0 - take, b)
            plan[ri][c] = (b, t0 - take, take)
    left = sum(t for t, b in rem)
    return (pad if left == 0 else None), plan


def _plan(context_lens):
    """Choose shared range sizes + (core, range) -> (seq, window) assignment.

    Padding is exactly NCORES*sum(rs) - total for any feasible plan, so
    search ascending per-core totals and take the first feasible config.
    """
    tiles = [int(math.ceil(int(c) / P)) for c in context_lens]
    total = sum(tiles)
    lo = (total + NCORES - 1) // NCORES
    rng = random.Random(0)
    for tot in range(lo, lo + 13):
        for R in range(4, 11):
            if R > tot:
                break
            for _ in range(3000):
                cuts = sorted(rng.sample(range(1, tot), R - 1)) if R > 1 else []
                s = [b - a for a, b in zip([0] + cuts, cuts + [tot])]
                if max(s) > 32:
                    continue
                s.sort(reverse=True)
                pad, plan = _greedy_assign(s, tiles)
                if pad is not None:
                    # descending size order: big ranges stream while the
                    # pipeline is deep, small ranges make a cheap tail
                    pairs = sorted(zip(s, plan), key=lambda x: -x[0])
                    return [p[0] for p in pairs], [p[1] for p in pairs]
    raise AssertionError("range planner failed")


def _blocks(rs):
    """[(ri, g, tile_off_in_range, bt)], with a small lead block to prime."""
    out = []
    for ri, n in enumerate(rs):
        for g in range(2):
            bo = 0
            if ri == 0 and g == 0 and n > 2:
                out.append((ri, g, 0, 2))
                bo = 2
            while bo < n:
                bt = min(BT, n - bo)
                out.append((ri, g, bo, bt))
                bo += bt
    return out


def _quantize(x):
    absmax = np.abs(x).max(axis=-1)
    scale = np.where(absmax > 0.0, absmax / 127.0, 1.0).astype(np.float32)
    xq = np.clip(np.round(x / scale[..., None]), -127.0, 127.0).astype(np.int8)
    return xq, scale


def _pack_inputs(inputs, rs, plan, blocks):
    q = inputs["q"].reshape(B, NUM_HEADS, D).astype(np.float32)
    k = inputs["k"].reshape(B, KVH, D).astype(np.float32)
    v = inputs["v"].reshape(B, KVH, D).astype(np.float32)
    kc = np.ascontiguousarray(
        inputs["k_cache_q"].reshape(-1, KVH, D).astype(np.int8))
    vc = np.ascontiguousarray(
        inputs["v_cache_q"].reshape(-1, KVH, D).astype(np.int8))
    ks = np.ascontiguousarray(inputs["k_scale"].reshape(-1, KVH)).astype(np.float32)
    vs = np.ascontiguousarray(inputs["v_scale"].reshape(-1, KVH)).astype(np.float32)
    bt_tab = inputs["block_tables"]
    ctx = inputs["context_lens"]
    sm = inputs["slot_mapping"]

    # store_kvcache_int8: quantize the new token and scatter into the cache
    kq, ksn = _quantize(k)
    vq, vsn = _quantize(v)
    kc = kc.copy(); vc = vc.copy(); ks = ks.copy(); vs = vs.copy()
    kc[sm] = kq; vc[sm] = vq; ks[sm] = ksn; vs[sm] = vsn

    R = len(rs)
    RT = sum(rs)
    offs = np.concatenate([[0], np.cumsum(rs)])
    KSZ = sum(bt * P * 4 * D for (_, _, _, bt) in blocks)     # int8 elems
    VSZ = sum(bt * 4 * P * DV for (_, _, _, bt) in blocks)

    # gather + zero-pad each sequence once, globally
    kg_all = {}; vg_all = {}; ksg_all = {}; vsg_all = {}
    for b in range(B):
        nt = int(math.ceil(int(ctx[b]) / P)) * P
        cl = int(ctx[b])
        flat = (bt_tab[b][:, None] * BLOCK_SIZE
                + np.arange(BLOCK_SIZE, dtype=np.int64)[None, :]).reshape(-1)[:nt]
        valid = (np.arange(nt) < cl)
        kg_all[b] = kc[flat] * valid[:, None, None]          # [nt, KVH, D]
        vg = vc[flat] * valid[:, None, None]
        n = nt // P
        vgm = np.zeros((n, P, KVH, DV), dtype=np.int8)
        vgm[:, :, :, :D] = vg.reshape(n, P, KVH, D)
        vgm[:, :, :, D] = valid.reshape(n, P)[:, :, None]
        vg_all[b] = vgm
        ksg_all[b] = (ks[flat] * SCALE) * valid[:, None]     # [nt, KVH]
        vsg_all[b] = vs[flat] * valid[:, None]

    in_maps = []
    for c in range(NCORES):
        kt_c = np.zeros((P, KSZ // P), dtype=np.int8)   # [d, flat]
        vp_c = np.zeros((P, VSZ // P), dtype=np.int8)   # [tok%128, flat]
        scb_c = np.zeros((P, 2, RT, 8), dtype=np.float32)
        qt_c = np.zeros((P, R * 32), dtype=BF16)
        # stage per-range gathered windows (padded to rs[ri] tiles)
        kw = {}; vw = {}
        for ri in range(R):
            n = rs[ri]
            o = int(offs[ri])
            w = plan[ri][c]
            kwin = np.zeros((n * P, KVH, D), dtype=np.int8)
            vwin = np.zeros((n, P, KVH, DV), dtype=np.int8)
            if w is not None:
                b, w0, take = w
                kwin[: take * P] = kg_all[b][w0 * P: (w0 + take) * P]
                vwin[: take] = vg_all[b][w0: w0 + take]
                ksgw = np.zeros((n * P, KVH), dtype=np.float32)
                vsgw = np.zeros((n * P, KVH), dtype=np.float32)
                ksgw[: take * P] = ksg_all[b][w0 * P: (w0 + take) * P]
                vsgw[: take * P] = vsg_all[b][w0 * P: (w0 + take) * P]
                scb_c[:, :, o: o + n, 0:4] = (
                    ksgw.reshape(n, P, 2, 4).transpose(1, 2, 0, 3))
                scb_c[:, :, o: o + n, 4:8] = (
                    vsgw.reshape(n, P, 2, 4).transpose(1, 2, 0, 3))
                qt_c[:, ri * 32: (ri + 1) * 32] = q[b].transpose(1, 0).astype(BF16)
            kw[ri] = kwin
            vw[ri] = vwin
        ko = vo = 0
        for (ri, g, bo, bt) in blocks:
            t0, t1 = bo * P, (bo + bt) * P
            kb = kw[ri][t0:t1, 4 * g: 4 * g + 4, :].transpose(2, 1, 0)  # [D,4,btP]
            ksz = 4 * bt * P
            kt_c[:, ko: ko + ksz] = kb.reshape(D, ksz)
            ko += ksz
            vb = vw[ri][bo: bo + bt, :, 4 * g: 4 * g + 4, :].transpose(1, 2, 0, 3)
            vsz = 4 * bt * DV
            vp_c[:, vo: vo + vsz] = vb.reshape(P, vsz)
            vo += vsz
        in_maps.append(dict(kt=kt_c, vp=vp_c, scb=scb_c, qt=qt_c))
    return in_maps


# ---------------------------------------------------------------------------
# device program
# ---------------------------------------------------------------------------

def _build_program(rs):
    blocks = _blocks(rs)
    R = len(rs)
    RT = sum(rs)
    offs = [0]
    for n in rs:
        offs.append(offs[-1] + n)
    KSZ = sum(bt * P * 4 * D for (_, _, _, bt) in blocks)
    VSZ = sum(bt * 4 * P * DV for (_, _, _, bt) in blocks)
    f32 = mybir.dt.float32
    bf16 = mybir.dt.bfloat16
    i8 = mybir.dt.int8
    EXP = mybir.ActivationFunctionType.Exp

    nc = bacc.Bacc("TRN2", target_bir_lowering=False, debug=False,
                   num_devices=NCORES)

    kt_d = nc.dram_tensor("kt", [P, KSZ // P], i8, kind="ExternalInput").ap()
    vp_d = nc.dram_tensor("vp", [P, VSZ // P], i8, kind="ExternalInput").ap()
    scb_d = nc.dram_tensor("scb", [P, 2, RT, 8], f32, kind="ExternalInput").ap()
    qt_d = nc.dram_tensor("qt", [P, R * 32], bf16, kind="ExternalInput").ap()
    out_d = nc.dram_tensor("out", [R, 2, P, DV], f32,
                           kind="ExternalOutput").ap()

    with tile.TileContext(nc) as tc, ExitStack() as ctx:
        const = ctx.enter_context(tc.tile_pool(name="const", bufs=1))
        kt_raw = ctx.enter_context(tc.tile_pool(name="ktr", bufs=4))
        v_raw = ctx.enter_context(tc.tile_pool(name="vpr", bufs=4))
        kt_pool = ctx.enter_context(tc.tile_pool(name="ktp", bufs=4))
        v_pool = ctx.enter_context(tc.tile_pool(name="vpp", bufs=4))
        sc_pool = ctx.enter_context(tc.tile_pool(name="scp", bufs=2))
        work = ctx.enter_context(tc.tile_pool(name="wrk", bufs=3))
        o_pool = ctx.enter_context(tc.tile_pool(name="osb", bufs=2))
        ps_qk = ctx.enter_context(tc.tile_pool(name="psqk", bufs=3, space="PSUM"))
        ps_pv = ctx.enter_context(tc.tile_pool(name="pspv", bufs=2, space="PSUM"))

        qt = const.tile([P, R * 32], bf16)
        nc.sync.dma_start(qt, qt_d)

        ko = vo = 0
        cur = None
        pv = scb = None
        blk_i = 0
        nblk = {}
        for (s2, g2, _, _) in blocks:
            nblk[(s2, g2)] = nblk.get((s2, g2), 0) + 1
        for (ri, g, bo, bt) in blocks:
            n = rs[ri]
            o = offs[ri]
            if cur != (ri, g):
                cur = (ri, g)
                scb = sc_pool.tile([P, n, 8], f32, tag="scb")
                nc.sync.dma_start(scb, scb_d[:, g, o: o + n, :])
                pv = ps_pv.tile([P, DV], f32, tag="pv")
                nc.vector.memset(pv, 0.0)
                blk_i = 0
            ksz = 4 * bt * P
            kcr = kt_raw.tile([P, 4, bt, P], i8, tag="ktr")
            nc.sync.dma_start(
                kcr, kt_d[:, ko: ko + ksz].rearrange(
                    "d (j i t) -> d j i t", j=4, i=bt))
            ko += ksz
            vsz = 4 * bt * DV
            vcr = v_raw.tile([P, 4, bt, DV], i8, tag="vpr")
            nc.scalar.dma_start(
                vcr, vp_d[:, vo: vo + vsz].rearrange(
                    "p (j i c) -> p j i c", j=4, i=bt))
            vo += vsz
            # on-chip int8 -> bf16 casts (DMA fabric stays at 1B/elem each way)
            kc = kt_pool.tile([P, 4, bt, P], bf16, tag="kt")
            nc.vector.tensor_copy(kc, kcr)
            vc = v_pool.tile([P, 4, bt, DV], bf16, tag="vt")
            nc.scalar.activation(vc, vcr, mybir.ActivationFunctionType.Copy)

            qk = ps_qk.tile([P, bt, 4, 4], f32, tag="qk")
            for i in range(bt):
                for j2 in range(4):
                    qcol = ri * 32 + (4 * g + j2) * 4
                    nc.tensor.matmul(
                        qk[:, i, j2, :],
                        lhsT=kc[:, j2, i, :],
                        rhs=qt[:, qcol: qcol + 4],
                        start=True, stop=True, skip_group_check=True)

            nc.vector.tensor_mul(
                qk, qk,
                scb[:, bo: bo + bt, 0:4].unsqueeze(3).to_broadcast(
                    [P, bt, 4, 4]))
            ew = work.tile([P, bt, 4, 8], bf16, tag="ew")
            nc.scalar.activation(ew[:, :, :, 0:4], qk, EXP)
            nc.vector.tensor_mul(
                ew[:, :, :, 4:8], ew[:, :, :, 0:4],
                scb[:, bo: bo + bt, 4:8].unsqueeze(3).to_broadcast(
                    [P, bt, 4, 4]))

            last = blk_i == nblk[cur] - 1
            for i in range(bt):
                for j2 in range(4):
                    nc.tensor.matmul(
                        pv[32 * j2: 32 * j2 + 8, :],
                        lhsT=ew[:, i, j2, :],
                        rhs=vc[:, j2, i, :],
                        start=(blk_i == 0 and i == 0),
                        stop=(last and i == bt - 1),
                        tile_position=(0, 32 * j2),
                        skip_group_check=True)
            blk_i += 1
            if last:
                osb = o_pool.tile([P, DV], f32, tag="osb")
                nc.vector.tensor_copy(osb, pv)
                nc.sync.dma_start(out_d[ri, g], osb)

    nc.compile()
    return nc


_PROGRAM_CACHE = {}
_PLAN_CACHE = {}


def _get_program(rs):
    key = tuple(rs)
    if key not in _PROGRAM_CACHE:
        _PROGRAM_CACHE[key] = _build_program(rs)
    return _PROGRAM_CACHE[key]


# ---------------------------------------------------------------------------
# entry point
# ---------------------------------------------------------------------------

def kernel(q, k, v, k_cache_q, v_cache_q, k_scale, v_scale,
           block_tables, context_lens, slot_mapping, _trace=False):
    inputs = dict(q=np.asarray(q), k=np.asarray(k), v=np.asarray(v),
                  k_cache_q=np.asarray(k_cache_q),
                  v_cache_q=np.asarray(v_cache_q),
                  k_scale=np.asarray(k_scale), v_scale=np.asarray(v_scale),
                  block_tables=np.asarray(block_tables),
                  context_lens=np.asarray(context_lens),
                  slot_mapping=np.asarray(slot_mapping))
    ctx_key = inputs["context_lens"].tobytes()
    if ctx_key not in _PLAN_CACHE:
        _PLAN_CACHE[ctx_key] = _plan(inputs["context_lens"])
    rs, plan = _PLAN_CACHE[ctx_key]
    blocks = _blocks(rs)
    in_maps = _pack_inputs(inputs, rs, plan, blocks)
    nc = _get_program(rs)
    res = run_bass_kernel_spmd(nc, in_maps, core_ids=list(range(NCORES)),
                               trace=_trace)

    num = np.zeros((B, NUM_HEADS, D), dtype=np.float64)
    den = np.zeros((B, NUM_HEADS), dtype=np.float64)
    R = len(rs)
    for c in range(NCORES):
        oc = res.results[c]["out"]  # [R, 2, P, DV] f32
        for ri in range(R):
            w = plan[ri][c]
            if w is None:
                continue
            b = w[0]
            for g in range(2):
                for j2 in range(4):
                    j = 4 * g + j2
                    den[b, 4 * j: 4 * j + 4] += oc[ri, g, 32 * j2: 32 * j2 + 4, D]
                    num[b, 4 * j: 4 * j + 4] += oc[ri, g,
                                                   32 * j2 + 4: 32 * j2 + 8, :D]
    out = (num / den[:, :, None]).astype(np.float32).reshape(B, NUM_HEADS * D)
    if _trace:
        return out, res
    return out
